# revision 1
# baseline (speedup 1.0000x reference)
"""Trainium2 Bass kernel for nn_CrossIQ (3-modality cross attention).

Reference computation (per batch b, with x0=rgb, x1=thermal, x2=depth):
    q_i = (wq_i @ x_i + bq_i) * s_i ; k_i likewise * s_i ; v_i = wv_i @ x_i + bv_i
    attend(q, ka, kb, v): softmax((q^T (ka+kb)) / 16, axis=m) -> a; out = v @ a^T
    w_rgb = attend(q0, k2, k1, v0); w_depth = attend(q2, k0, k1, v2)
    w_thermal = attend(q1, k0, k2, v1)
    fused = sum_i sigmoid(wg @ w_i + bg) * w_i ;  out = wo @ fused + bo

Sharding: 8 cores = (batch b in 0..3) x (query-half h in 0..1). Attention is
independent per batch and per query row n, so each core computes the full
k/v projections for its batch but only its half of the score rows.

Device dataflow per core (all matmul operands in DT = float32r, fp32 PSUM):
  - ksum_a[c,m] = wk_beta @ x_beta + wk_gamma @ x_gamma + combined bias
    (single PSUM accumulation; s_i folded into weights on host)
  - q_a[c,n] from the first NH columns of x (the host rotates each core's
    query half to the front, so the SPMD program is identical on all cores);
    s_a/16 folded into wq
  - vT_a[m,c] = x_a^T @ wv_a^T + bias (transposed v so attention needs no
    on-device transposes)
  - scoresT[m,n] tiles = ksum^T q; pT = exp(scoresT)  (max-free softmax:
    scores are O(few) by construction, fp32 exp is safe)
  - attn_raw[c,n] += vT[m,:]^T @ pT ; denom[1,n] += ones^T @ pT
  - attn = attn_raw * reciprocal(denom) broadcast via ones-matmul
  - fused += sigmoid(wg attn + bg) * attn ; out = wo fused + bo
"""
import os
import sys
import types
import numpy as np
import ml_dtypes

# --- defensive shim: antenv.axon_hooks may be absent in this image; concourse
# imports it when trace=True under axon. Harmless no-op registration.
try:
    import antenv  # noqa: F401
    if "antenv.axon_hooks" not in sys.modules:
        _m = types.ModuleType("antenv.axon_hooks")
        _m._hook = None
        def _set(h):
            _m._hook = h
        def _get():
            return _m._hook
        _m.set_axon_ntff_profile_hook = _set
        _m.get_axon_ntff_profile_hook = _get
        sys.modules["antenv.axon_hooks"] = _m
        try:
            from trn_agent_boot.trn_boot import _ntff_profile_via_ctypes
            _h = _ntff_profile_via_ctypes("/opt/axon/libaxon_pjrt.so")
            if _h is not None:
                _m._hook = _h
        except Exception:
            pass
except Exception:
    pass

import concourse.bacc as bacc
import concourse.mybir as mybir
import concourse.tile as tile
from concourse.bass_utils import run_bass_kernel_spmd

B, C, H, W = 4, 256, 48, 48
N = H * W          # 2304 pixels (the m / key axis)
NH = N // 2        # 1152 query rows per core
NT = 384           # n-tile (<= 512 fp32 PSUM bank)
NNT = NH // NT     # 3 n-tiles
MC = N // 128      # 18 m-chunks
MT = 384           # m-tile for the ksum conv
NMT = N // MT      # 6
SCALE = 16.0       # sqrt(C)

DT_NAME = os.environ.get("KERNEL_DT", "float32r")

LAST_EXEC_NS = None
LAST_RESULTS = None

_CACHE = {}


def _dt():
    return getattr(mybir.dt, DT_NAME)


def _np_dt():
    return mybir.dt.np(_dt())


def build_bass():
    """Build the single-core program (identical on all 8 cores)."""
    DT = _dt()
    f32 = mybir.dt.float32
    nc = bacc.Bacc("TRN2", target_bir_lowering=False, debug=False)

    # ---- DRAM I/O ----
    xs_d = [nc.dram_tensor(f"x{i}", [128, 2, N], DT, kind="ExternalInput").ap()
            for i in range(3)]
    wq_d = [nc.dram_tensor(f"wqT{i}", [128, 2, C], DT, kind="ExternalInput").ap()
            for i in range(3)]
    wk_d = [nc.dram_tensor(f"wkT{i}", [128, 2, C], DT, kind="ExternalInput").ap()
            for i in range(3)]
    wv_d = [nc.dram_tensor(f"wvT{i}", [128, 2, C], DT, kind="ExternalInput").ap()
            for i in range(3)]
    wg_d = nc.dram_tensor("wgT", [128, 2, C], DT, kind="ExternalInput").ap()
    wo_d = nc.dram_tensor("woT", [128, 2, C], DT, kind="ExternalInput").ap()
    bq_d = [nc.dram_tensor(f"bq{i}", [128, 2], f32, kind="ExternalInput").ap()
            for i in range(3)]
    bks_d = [nc.dram_tensor(f"bks{a}", [128, 2], f32, kind="ExternalInput").ap()
             for a in range(3)]
    bv_d = nc.dram_tensor("bvb", [128, 3, C], f32, kind="ExternalInput").ap()
    bg_d = nc.dram_tensor("bgp", [128, 2], f32, kind="ExternalInput").ap()
    bo_d = nc.dram_tensor("bop", [128, 2], f32, kind="ExternalInput").ap()
    onc_d = nc.dram_tensor("ones_c", [128, 4], DT, kind="ExternalInput").ap()
    onr_d = nc.dram_tensor("ones_r", [1, 128], DT, kind="ExternalInput").ap()
    out_d = nc.dram_tensor("out", [128, 2, NH], f32, kind="ExternalOutput").ap()

    # attention spec: (alpha = q/v modality, beta/gamma = key modalities)
    ATTN = [(0, 2, 1), (2, 0, 1), (1, 0, 2)]

    with tile.TileContext(nc) as tc:
        with (
            tc.tile_pool(name="consts", bufs=1) as consts,
            tc.tile_pool(name="ksum_p", bufs=2) as ksum_p,
            tc.tile_pool(name="q_p", bufs=2) as q_p,
            tc.tile_pool(name="vt_p", bufs=2) as vt_p,
            tc.tile_pool(name="pt_p", bufs=8) as pt_p,
            tc.tile_pool(name="attn_p", bufs=1) as attn_p,
            tc.tile_pool(name="small_p", bufs=1) as small_p,
            tc.tile_pool(name="eplg_p", bufs=1) as eplg_p,
            tc.tile_pool(name="out_p", bufs=1) as out_p,
            tc.tile_pool(name="pp_conv", bufs=2, space="PSUM") as pp_conv,
            tc.tile_pool(name="pp_sc", bufs=3, space="PSUM") as pp_sc,
            tc.tile_pool(name="pp_at", bufs=1, space="PSUM") as pp_at,
            tc.tile_pool(name="pp_dn", bufs=1, space="PSUM") as pp_dn,
        ):
            # ---- load constants (small weights/biases first, then x in
            # chunks so the first convs can start early) ----
            def w_load(tag, src_ap, eng=None):
                eng = eng or nc.sync
                t = consts.tile([128, 2, C], DT, tag=tag, name=f"{tag}_sb")
                for st in range(2):
                    pr = slice(st * 64, (st + 1) * 64)
                    eng.dma_start(out=t[pr, :, :], in_=src_ap[pr, :, :])
                return t

            def b_load(tag, src_ap):
                t = consts.tile([128, 2], f32, tag=tag, name=f"{tag}_sb")
                nc.sync.dma_start(out=t[:], in_=src_ap[:])
                return t

            # priority order: what attention 0's phase A touches first —
            # wk2/wk1 + x2/x1 (its ksum), then x0/wq0/wv0, then the rest
            wks = [None, None, None]
            wqs = [None, None, None]
            wvs = [None, None, None]
            bqs = [None, None, None]
            bkss = [None, None, None]
            x_tiles = {}

            for i in range(3):
                for ci in range(2):
                    x_tiles[(i, ci)] = consts.tile(
                        [128, N], DT, tag=f"x{i}_{ci}", name=f"x_sb{i}_{ci}")

            wks[2] = w_load("wk2", wk_d[2])
            wks[1] = w_load("wk1", wk_d[1])
            bkss[0] = b_load("bks0", bks_d[0])
            # column-chunked full-partition DMAs (partition-strips would
            # waste SBUF write ports). Small leading chunks for a fast first
            # conv, wide trailing chunks for descriptor efficiency; chunks
            # alternate between the HW-DGE (sync) and SW-DGE (gpsimd) queue
            # sets to use all 16 DMA engines. x2/x1 first.
            def x_load(i, ci):
                for k, (lo, hi) in enumerate(((0, 576), (576, 1152), (1152, 2304))):
                    eng = nc.sync if (k + ci) % 2 == 0 else nc.gpsimd
                    eng.dma_start(out=x_tiles[(i, ci)][:, lo:hi],
                                  in_=xs_d[i][:, ci, lo:hi])
            x_load(2, 0)
            x_load(2, 1)
            x_load(1, 0)
            x_load(1, 1)
            wqs[0] = w_load("wq0", wq_d[0])
            bqs[0] = b_load("bq0", bq_d[0])
            wvs[0] = w_load("wv0", wv_d[0])
            x_load(0, 0)
            x_load(0, 1)
            wks[0] = w_load("wk0", wk_d[0], eng=nc.gpsimd)
            for i in (1, 2):
                wqs[i] = w_load(f"wq{i}", wq_d[i], eng=nc.gpsimd)
                wvs[i] = w_load(f"wv{i}", wv_d[i], eng=nc.gpsimd)
                bqs[i] = b_load(f"bq{i}", bq_d[i])
            for a in (1, 2):
                bkss[a] = b_load(f"bks{a}", bks_d[a])
            wg_sb = w_load("wg", wg_d, eng=nc.gpsimd)
            wo_sb = w_load("wo", wo_d, eng=nc.gpsimd)
            bv_sb = consts.tile([128, 3, C], f32, tag="bv")
            nc.sync.dma_start(out=bv_sb[:], in_=bv_d[:])
            bgn_sb = consts.tile([128, 2], f32, tag="bgn")
            nc.sync.dma_start(out=bgn_sb[:], in_=bg_d[:])
            bo_sb = consts.tile([128, 2], f32, tag="bo")
            nc.sync.dma_start(out=bo_sb[:], in_=bo_d[:])

            ones_col = consts.tile([128, 4], DT, tag="ones_col")
            nc.sync.dma_start(out=ones_col[:], in_=onc_d[:])
            ones_row = consts.tile([1, 128], DT, tag="ones_row")
            nc.sync.dma_start(out=ones_row[:], in_=onr_d[:])

            def xsl(i, ci, cols):
                return x_tiles[(i, ci)][:, cols]

            deferred = []

            def flush_deferred():
                while deferred:
                    deferred.pop(0)()

            fused = consts.tile([128, 2, NH], f32, tag="fused")
            nc.vector.memset(fused[:], 0.0)

            def phase_a(a):
                al, be, ga = ATTN[a]
                scopeA = nc.named_scope(f"phA_{a}"); scopeA.__enter__()
                ksum = ksum_p.tile([128, 2, N], DT, tag="ksum", name=f"ksum{a}")
                for co in range(2):
                    for mt in range(NMT):
                        pk = pp_conv.tile([128, MT], f32, tag="convp", name=f"pk{a}_{co}_{mt}")
                        mcols = slice(mt * MT, (mt + 1) * MT)
                        for idx, mod in enumerate((be, ga)):
                            for ci in range(2):
                                nc.tensor.matmul(
                                    pk[:],
                                    wks[mod][:, ci, co * 128:(co + 1) * 128],
                                    xsl(mod, ci, mcols),
                                    start=(idx == 0 and ci == 0),
                                    stop=(idx == 1 and ci == 1),
                                )
                        nc.vector.tensor_scalar_add(
                            ksum[:, co, mcols], pk[:], bkss[a][:, co:co + 1])

                q_sb = q_p.tile([128, 2, NH], DT, tag="q", name=f"q{a}")
                for co in range(2):
                    for nt in range(NNT):
                        pq = pp_conv.tile([128, NT], f32, tag="convp", name=f"pq{a}_{co}_{nt}")
                        ncols = slice(nt * NT, (nt + 1) * NT)
                        for ci in range(2):
                            nc.tensor.matmul(
                                pq[:],
                                wqs[al][:, ci, co * 128:(co + 1) * 128],
                                xsl(al, ci, ncols),
                                start=(ci == 0), stop=(ci == 1),
                            )
                        nc.vector.tensor_scalar_add(
                            q_sb[:, co, ncols], pq[:], bqs[al][:, co:co + 1])

                vt = vt_p.tile([128, MC, C], DT, tag="vt", name=f"vt{a}")
                for m in range(MC):
                    pv = pp_conv.tile([128, C], f32, tag="convp", name=f"pv{a}_{m}")
                    for ci in range(2):
                        nc.tensor.matmul(
                            pv[:],
                            xsl(al, ci, slice(m * 128, (m + 1) * 128)),
                            wvs[al][:, ci, :],
                            start=(ci == 0), stop=(ci == 1),
                        )
                    nc.vector.tensor_add(vt[:, m, :], pv[:], bv_sb[:, al, :])

                scopeA.__exit__(None, None, None)
                return ksum, q_sb, vt

            # ---- output conv (interleaved: called per-nt once the last
            # attention's epilogue for that column range has flushed) ----
            def out_conv(nt):
                ncols = slice(nt * NT, (nt + 1) * NT)
                fdt = out_p.tile([128, 2, NT], DT, tag="fdt", name=f"fdt{nt}")
                nc.vector.tensor_copy(fdt[:, 0, :], fused[:, 0, ncols])
                nc.vector.tensor_copy(fdt[:, 1, :], fused[:, 1, ncols])
                for co in range(2):
                    po = pp_conv.tile([128, NT], f32, tag="convp", name=f"po{nt}_{co}")
                    for ci in range(2):
                        nc.tensor.matmul(
                            po[:],
                            wo_sb[:, ci, co * 128:(co + 1) * 128],
                            fdt[:, ci, :],
                            start=(ci == 0), stop=(ci == 1),
                        )
                    osb = out_p.tile([128, NT], f32, tag="osb", name=f"osb{nt}_{co}")
                    nc.vector.tensor_scalar_add(osb[:], po[:], bo_sb[:, co:co + 1])
                    nc.sync.dma_start(out=out_d[:, co, ncols], in_=osb[:])


            proj = phase_a(0)
            for a, (al, be, ga) in enumerate(ATTN):
                ksum, q_sb, vt = proj
                # ---- phase B: attention over n-tiles ----
                for nt in range(NNT):
                    if nt == NNT - 1 and a < 2:
                        proj = phase_a(a + 1)
                    scopeB = nc.named_scope(f"phB_{a}_{nt}"); scopeB.__enter__()
                    flush_deferred()
                    if a == 2 and nt >= 1:
                        out_conv(nt - 1)
                    ncols = slice(nt * NT, (nt + 1) * NT)
                    at0 = pp_at.tile([128, NT], f32, tag="at0", name=f"at0_{a}_{nt}")
                    at1 = pp_at.tile([128, NT], f32, tag="at1", name=f"at1_{a}_{nt}")
                    dn = pp_dn.tile([1, NT], f32, tag="dn", name=f"dn{a}_{nt}")
                    # scores run LEAD chunks ahead of the accumulation
                    # passes: the lead hides the previous tile's epilogue
                    # chain (at-bank release) and the exp latency
                    LEAD = 6
                    pts = {}
                    for m in range(MC + LEAD):
                        if m < MC:
                            sc_ps = pp_sc.tile([128, NT], f32, tag="sc", name=f"sc{a}_{nt}_{m}")
                            for ci in range(2):
                                nc.tensor.matmul(
                                    sc_ps[:],
                                    ksum[:, ci, m * 128:(m + 1) * 128],
                                    q_sb[:, ci, ncols],
                                    start=(ci == 0), stop=(ci == 1),
                                )
                            pt = pt_p.tile([128, NT], DT, tag="pt", name=f"pt{a}_{nt}_{m}")
                            nc.scalar.activation(
                                pt[:], sc_ps[:], mybir.ActivationFunctionType.Exp)
                            pts[m] = pt
                        if m >= LEAD:
                            j = m - LEAD
                            nc.tensor.matmul(at0[:], vt[:, j, 0:128], pts[j][:],
                                             start=(j == 0), stop=(j == MC - 1))
                            nc.tensor.matmul(at1[:], vt[:, j, 128:256], pts[j][:],
                                             start=(j == 0), stop=(j == MC - 1))
                            nc.tensor.matmul(dn[:], ones_col[:, j % 4:j % 4 + 1], pts[j][:],
                                             start=(j == 0), stop=(j == MC - 1))

                    # epilogue is deferred until after the NEXT scores pass
                    # is emitted, so the PE chews on those scores while this
                    # DVE/ACT chain drains; the at banks release just before
                    # the next tile's accumulation passes need them
                    r_sb = small_p.tile([1, NT], f32, tag="r", name=f"r{a}_{nt}")
                    nc.vector.reciprocal_approx_fast(r_sb[:], dn[:])

                    def epilogue(a=a, nt=nt, ncols=ncols, at0=at0, at1=at1, r_sb=r_sb):
                        rdt = small_p.tile([1, NT], DT, tag="rdt", name=f"rdt{a}_{nt}")
                        nc.vector.tensor_copy(rdt[:], r_sb[:])
                        rb = pp_conv.tile([128, NT], f32, tag="convp", name=f"rb{a}_{nt}")
                        nc.tensor.matmul(rb[:], ones_row[:], rdt[:])
                        rb_sb = small_p.tile([128, NT], f32, tag="rb_sb", name=f"rbs{a}_{nt}")
                        nc.vector.tensor_copy(rb_sb[:], rb[:])
                        attn = attn_p.tile([128, 2, NT], DT, tag="attn", name=f"attn{a}_{nt}")
                        nc.vector.tensor_mul(attn[:, 0, :], at0[:], rb_sb[:])
                        nc.vector.tensor_mul(attn[:, 1, :], at1[:], rb_sb[:])
                        # gate: fused += sigmoid(wg attn + bg) * attn
                        for co in range(2):
                            pg = pp_conv.tile([128, NT], f32, tag="convp", name=f"pg{a}_{nt}_{co}")
                            for ci in range(2):
                                nc.tensor.matmul(
                                    pg[:],
                                    wg_sb[:, ci, co * 128:(co + 1) * 128],
                                    attn[:, ci, :],
                                    start=(ci == 0), stop=(ci == 1),
                                )
                            # sigmoid(pg+bg) = 1/(1+exp(-pg-bg)); Exp keeps
                            # the ACT engine on a single function table
                            en = eplg_p.tile([128, NT], f32, tag="en", name=f"en{a}_{nt}_{co}")
                            nc.scalar.activation(
                                en[:], pg[:], mybir.ActivationFunctionType.Exp,
                                bias=bgn_sb[:, co:co + 1], scale=-1.0)
                            nc.vector.tensor_scalar_add(en[:], en[:], 1.0)
                            nc.vector.reciprocal_approx_fast(en[:], en[:])
                            gt = eplg_p.tile([128, NT], f32, tag="gt", name=f"gt{a}_{nt}_{co}")
                            nc.vector.tensor_mul(gt[:], en[:], attn[:, co, :])
                            nc.vector.tensor_add(
                                fused[:, co, ncols], fused[:, co, ncols], gt[:])

                    scopeB.__exit__(None, None, None)
                    deferred.append(epilogue)

            flush_deferred()
            out_conv(2)

    nc.compile()
    return nc


def _pack_chw(arr):
    """[256, X] f32 -> [128, 2, X] in DT order (c_lo, c_hi, X)."""
    return np.ascontiguousarray(
        arr.reshape(2, 128, -1).transpose(1, 0, 2)).astype(_np_dt())


def _pack_bias(b):
    """[256] -> [128, 2] f32 (c_lo, c_hi)."""
    return np.ascontiguousarray(b.reshape(2, 128).T).astype(np.float32)


def _pack_w(w, scale=1.0):
    """[c_out, c_in] -> lhsT layout [128, 2, 256] = (c_in_lo, c_in_hi, c_out)."""
    wt = (w.astype(np.float64) * scale).astype(np.float32).T  # [c_in, c_out]
    return np.ascontiguousarray(
        wt.reshape(2, 128, C).transpose(1, 0, 2)).astype(_np_dt())


def kernel(**inputs):
    global LAST_EXEC_NS, LAST_RESULTS
    inp = {k: np.asarray(v) for k, v in inputs.items()}
    s = inp["s"].astype(np.float32)

    if "nc" not in _CACHE:
        _CACHE["nc"] = build_bass()
    nc = _CACHE["nc"]

    # ---- host-side packing ----
    ATTN = [(0, 2, 1), (2, 0, 1), (1, 0, 2)]
    shared = {}
    for i in range(3):
        shared[f"wqT{i}"] = _pack_w(inp[f"wq{i}"], s[i] / SCALE)
        shared[f"wkT{i}"] = _pack_w(inp[f"wk{i}"], s[i])
        shared[f"wvT{i}"] = _pack_w(inp[f"wv{i}"])
        shared[f"bq{i}"] = _pack_bias(inp[f"bq{i}"].astype(np.float32) * (s[i] / SCALE))
    for a, (al, be, ga) in enumerate(ATTN):
        shared[f"bks{a}"] = _pack_bias(
            inp[f"bk{be}"].astype(np.float32) * s[be]
            + inp[f"bk{ga}"].astype(np.float32) * s[ga])
    shared["wgT"] = _pack_w(inp["wg"])
    shared["woT"] = _pack_w(inp["wo"])
    bv = np.stack([np.tile(inp[f"bv{i}"].astype(np.float32)[None, :], (128, 1))
                   for i in range(3)], axis=1)  # [128, 3, 256]
    shared["bvb"] = np.ascontiguousarray(bv)
    shared["bgp"] = _pack_bias(-inp["bg"].astype(np.float32))
    shared["ones_c"] = np.ones((128, 4), _np_dt())
    shared["ones_r"] = np.ones((1, 128), _np_dt())
    shared["bop"] = _pack_bias(inp["bo"])

    in_maps = []
    for core in range(8):
        b, h = core // 2, core % 2
        m = dict(shared)
        for i in range(3):
            xp = _pack_chw(inp[f"x{i}"][b].reshape(C, N).astype(np.float32))
            if h == 1:  # rotate so this core's query half comes first
                xp = np.ascontiguousarray(
                    np.concatenate([xp[:, :, NH:], xp[:, :, :NH]], axis=2))
            m[f"x{i}"] = xp
        in_maps.append(m)

    trace = bool(os.environ.get("BASS_TRACE"))
    res = run_bass_kernel_spmd(nc, in_maps, core_ids=list(range(8)), trace=trace)
    LAST_EXEC_NS = res.exec_time_ns
    LAST_RESULTS = res

    out = np.empty((B, C, N), np.float32)
    for core in range(8):
        b, h = core // 2, core % 2
        o = res.results[core]["out"]  # [128, 2, NH] f32
        out[b, :, h * NH:(h + 1) * NH] = o.transpose(1, 0, 2).reshape(C, NH)
    return out.reshape(B, C, H, W)



# revision 2
# speedup vs baseline: 1.0252x; 1.0252x over previous
"""Trainium2 Bass kernel for nn_CrossIQ (3-modality cross attention).

Reference computation (per batch b, with x0=rgb, x1=thermal, x2=depth):
    q_i = (wq_i @ x_i + bq_i) * s_i ; k_i likewise * s_i ; v_i = wv_i @ x_i + bv_i
    attend(q, ka, kb, v): softmax((q^T (ka+kb)) / 16, axis=m) -> a; out = v @ a^T
    w_rgb = attend(q0, k2, k1, v0); w_depth = attend(q2, k0, k1, v2)
    w_thermal = attend(q1, k0, k2, v1)
    fused = sum_i sigmoid(wg @ w_i + bg) * w_i ;  out = wo @ fused + bo

Sharding: 8 cores = (batch b in 0..3) x (query-half h in 0..1). Attention is
independent per batch and per query row n, so each core computes the full
k/v projections for its batch but only its half of the score rows.

v2 device dataflow per core (all matmul operands bf16, fp32 PSUM):
  - k_i[c,m] = wk_i @ x_i + bk_i*s_i computed ONCE per modality (72 PE passes
    instead of 144); ksum_a = k_be + k_ga via DVE pair-sum (2-byte 2x mode)
  - v bias is dropped on device: softmax weights sum to 1, so
    attend(q,ka,kb,v+bv) = attend(q,ka,kb,v) + bv. bv is re-added in the
    epilogue (0.5*bv onto the halved attn) and wg@bv is NOT needed because
    the epilogue adds bv before the gate conv.
  - scoresT[m,n] tiles = ksum^T q; pT = exp(scoresT) bf16 (max-free softmax)
  - attn_raw[c,n] += vT[m,:]^T @ pT ; denom[1,n] += ones^T @ pT
  - attn_h = attn_raw * (0.5*reciprocal(denom)) + 0.5*bv   (halved attention)
  - gate via tanh (same ACT table as Exp, so no 1.28us table reloads):
    sigma = 0.5*(1+tanh((wg@attn+bg)/2)); with wg2 = 2*wg on host,
    pg = wg2@attn_h = wg@attn, th = tanh(0.5*pg + bg/2)
    fused += attn_h + th*attn_h  ( = sigma * attn )
  - out = wo fused + bo, written fp16 (values <<1, fp16 exact to ~5e-4)
"""
import os
import sys
import types
import numpy as np
import ml_dtypes

# --- defensive shim: antenv.axon_hooks may be absent in this image; concourse
# imports it when trace=True under axon. Harmless no-op registration.
try:
    import antenv  # noqa: F401
    if "antenv.axon_hooks" not in sys.modules:
        _m = types.ModuleType("antenv.axon_hooks")
        _m._hook = None
        def _set(h):
            _m._hook = h
        def _get():
            return _m._hook
        _m.set_axon_ntff_profile_hook = _set
        _m.get_axon_ntff_profile_hook = _get
        sys.modules["antenv.axon_hooks"] = _m
        try:
            from trn_agent_boot.trn_boot import _ntff_profile_via_ctypes
            _h = _ntff_profile_via_ctypes("/opt/axon/libaxon_pjrt.so")
            if _h is not None:
                _m._hook = _h
        except Exception:
            pass
except Exception:
    pass

import concourse.bacc as bacc
import concourse.mybir as mybir
import concourse.tile as tile
from concourse.bass_utils import run_bass_kernel_spmd

B, C, H, W = 4, 256, 48, 48
N = H * W          # 2304 pixels (the m / key axis)
NH = N // 2        # 1152 query rows per core
NT = 384           # n-tile (<= 512 fp32 PSUM bank)
NNT = NH // NT     # 3 n-tiles
MC = N // 128      # 18 m-chunks
MT = 384           # m-tile for the k convs
NMT = N // MT      # 6
SCALE = 16.0       # sqrt(C)

DT_NAME = os.environ.get("KERNEL_DT", "bfloat16")

LAST_EXEC_NS = None
LAST_RESULTS = None

_CACHE = {}


def _dt():
    return getattr(mybir.dt, DT_NAME)


def _np_dt():
    return mybir.dt.np(_dt())


def build_bass():
    """Build the single-core program (identical on all 8 cores)."""
    DT = _dt()
    f32 = mybir.dt.float32
    f16 = mybir.dt.float16
    nc = bacc.Bacc("TRN2", target_bir_lowering=False, debug=False)

    # ---- DRAM I/O ----
    xs_d = [nc.dram_tensor(f"x{i}", [128, 2, N], DT, kind="ExternalInput").ap()
            for i in range(3)]
    wq_d = [nc.dram_tensor(f"wqT{i}", [128, 2, C], DT, kind="ExternalInput").ap()
            for i in range(3)]
    wk_d = [nc.dram_tensor(f"wkT{i}", [128, 2, C], DT, kind="ExternalInput").ap()
            for i in range(3)]
    wv_d = [nc.dram_tensor(f"wvT{i}", [128, 2, C], DT, kind="ExternalInput").ap()
            for i in range(3)]
    wg_d = nc.dram_tensor("wgT", [128, 2, C], DT, kind="ExternalInput").ap()
    wo_d = nc.dram_tensor("woT", [128, 2, C], DT, kind="ExternalInput").ap()
    bq_d = [nc.dram_tensor(f"bq{i}", [128, 2], f32, kind="ExternalInput").ap()
            for i in range(3)]
    bk_d = [nc.dram_tensor(f"bk{i}", [128, 2], f32, kind="ExternalInput").ap()
            for i in range(3)]
    bvh_d = nc.dram_tensor("bvh", [128, 3, 2], f32, kind="ExternalInput").ap()
    bgh_d = nc.dram_tensor("bgh", [128, 2], f32, kind="ExternalInput").ap()
    bo_d = nc.dram_tensor("bop", [128, 2], f32, kind="ExternalInput").ap()
    onc_d = nc.dram_tensor("ones_c", [128, 4], DT, kind="ExternalInput").ap()
    onr_d = nc.dram_tensor("half_r", [1, 128], DT, kind="ExternalInput").ap()
    out_d = nc.dram_tensor("out", [128, 2, NH], f16, kind="ExternalOutput").ap()

    # attention spec: (alpha = q/v modality, beta/gamma = key modalities)
    ATTN = [(0, 2, 1), (2, 0, 1), (1, 0, 2)]

    with tile.TileContext(nc) as tc:
        with (
            tc.tile_pool(name="consts", bufs=1) as consts,
            tc.tile_pool(name="ksum_p", bufs=2) as ksum_p,
            tc.tile_pool(name="q_p", bufs=2) as q_p,
            tc.tile_pool(name="vt_p", bufs=2) as vt_p,
            tc.tile_pool(name="pt_p", bufs=8) as pt_p,
            tc.tile_pool(name="attn_p", bufs=1) as attn_p,
            tc.tile_pool(name="small_p", bufs=1) as small_p,
            tc.tile_pool(name="eplg_p", bufs=1) as eplg_p,
            tc.tile_pool(name="out_p", bufs=1) as out_p,
            tc.tile_pool(name="pp_conv", bufs=2, space="PSUM") as pp_conv,
            tc.tile_pool(name="pp_sc", bufs=3, space="PSUM") as pp_sc,
            tc.tile_pool(name="pp_at", bufs=1, space="PSUM") as pp_at,
            tc.tile_pool(name="pp_dn", bufs=1, space="PSUM") as pp_dn,
        ):
            # ---- load constants (small weights/biases first, then x in
            # chunks so the first convs can start early) ----
            def w_load(tag, src_ap, eng=None):
                eng = eng or nc.sync
                t = consts.tile([128, 2, C], DT, tag=tag, name=f"{tag}_sb")
                for st in range(2):
                    pr = slice(st * 64, (st + 1) * 64)
                    eng.dma_start(out=t[pr, :, :], in_=src_ap[pr, :, :])
                return t

            def b_load(tag, src_ap):
                t = consts.tile([128, 2], f32, tag=tag, name=f"{tag}_sb")
                nc.sync.dma_start(out=t[:], in_=src_ap[:])
                return t

            wks = [None, None, None]
            wqs = [None, None, None]
            wvs = [None, None, None]
            bqs = [None, None, None]
            bks = [None, None, None]
            x_tiles = {}
            for i in range(3):
                for ci in range(2):
                    x_tiles[(i, ci)] = consts.tile(
                        [128, N], DT, tag=f"x{i}_{ci}", name=f"x_sb{i}_{ci}")

            # k_i conv outputs, persistent (reused by 2 attentions each)
            k_sb = [consts.tile([128, 2, N], DT, tag=f"k{i}", name=f"k_sb{i}")
                    for i in range(3)]

            # priority: attention 0 needs ksum0 = k2+k1 (x2/x1) and q0 (x0
            # front cols). DMA x2/x1 in mt-granular chunks so the k convs
            # pipeline with the DMA; x0 follows.
            wks[2] = w_load("wk2", wk_d[2])
            wks[1] = w_load("wk1", wk_d[1])
            bks[2] = b_load("bk2", bk_d[2])
            bks[1] = b_load("bk1", bk_d[1])
            wqs[0] = w_load("wq0", wq_d[0])
            bqs[0] = b_load("bq0", bq_d[0])

            def x_load_chunk(i, ci, mt, eng):
                lo, hi = mt * MT, (mt + 1) * MT
                eng.dma_start(out=x_tiles[(i, ci)][:, lo:hi],
                              in_=xs_d[i][:, ci, lo:hi])

            for mt in range(NMT):
                x_load_chunk(2, 0, mt, nc.sync)
                x_load_chunk(2, 1, mt, nc.gpsimd)
                x_load_chunk(1, 0, mt, nc.sync)
                x_load_chunk(1, 1, mt, nc.gpsimd)
            wvs[0] = w_load("wv0", wv_d[0])
            for mt in range(NMT):
                x_load_chunk(0, 0, mt, nc.sync if mt % 2 else nc.gpsimd)
                x_load_chunk(0, 1, mt, nc.gpsimd if mt % 2 else nc.sync)
            wks[0] = w_load("wk0", wk_d[0], eng=nc.gpsimd)
            bks[0] = b_load("bk0", bk_d[0])
            for i in (1, 2):
                wqs[i] = w_load(f"wq{i}", wq_d[i], eng=nc.gpsimd)
                wvs[i] = w_load(f"wv{i}", wv_d[i], eng=nc.gpsimd)
                bqs[i] = b_load(f"bq{i}", bq_d[i])
            wg_sb = w_load("wg", wg_d, eng=nc.gpsimd)
            wo_sb = w_load("wo", wo_d, eng=nc.gpsimd)
            bvh_sb = consts.tile([128, 3, 2], f32, tag="bvh")
            nc.sync.dma_start(out=bvh_sb[:], in_=bvh_d[:])
            bgh_sb = consts.tile([128, 2], f32, tag="bgh")
            nc.sync.dma_start(out=bgh_sb[:], in_=bgh_d[:])
            bo_sb = consts.tile([128, 2], f32, tag="bo")
            nc.sync.dma_start(out=bo_sb[:], in_=bo_d[:])

            ones_col = consts.tile([128, 4], DT, tag="ones_col")
            nc.sync.dma_start(out=ones_col[:], in_=onc_d[:])
            half_row = consts.tile([1, 128], DT, tag="half_row")
            nc.sync.dma_start(out=half_row[:], in_=onr_d[:])

            def xsl(i, ci, cols):
                return x_tiles[(i, ci)][:, cols]

            deferred = []

            def flush_deferred():
                while deferred:
                    deferred.pop(0)()

            fused = consts.tile([128, 2, NH], DT, tag="fused")
            nc.vector.memset(fused[:], 0.0)

            # ---- modality k conv: k_i[:, co, mt-cols] (+bias via DVE) ----
            def k_conv_chunk(i, mt):
                mcols = slice(mt * MT, (mt + 1) * MT)
                for co in range(2):
                    pk = pp_conv.tile([128, MT], f32, tag="convp",
                                      name=f"pk{i}_{co}_{mt}")
                    for ci in range(2):
                        nc.tensor.matmul(
                            pk[:],
                            wks[i][:, ci, co * 128:(co + 1) * 128],
                            xsl(i, ci, mcols),
                            start=(ci == 0), stop=(ci == 1),
                        )
                    nc.vector.tensor_scalar_add(
                        k_sb[i][:, co, mcols], pk[:], bks[i][:, co:co + 1])

            # ---- pair sum ksum_a = k_be + k_ga (DVE, bf16 2x mode) ----
            def ksum_add(ksum, be, ga, mt_lo, mt_hi):
                mcols = slice(mt_lo * MT, mt_hi * MT)
                nc.vector.tensor_add(
                    ksum[:, :, mcols], k_sb[be][:, :, mcols],
                    k_sb[ga][:, :, mcols])

            # ---- q conv for one n-tile ----
            def q_conv_tile(a, al, q_sb, nt):
                ncols = slice(nt * NT, (nt + 1) * NT)
                for co in range(2):
                    pq = pp_conv.tile([128, NT], f32, tag="convp",
                                      name=f"pq{a}_{co}_{nt}")
                    for ci in range(2):
                        nc.tensor.matmul(
                            pq[:],
                            wqs[al][:, ci, co * 128:(co + 1) * 128],
                            xsl(al, ci, ncols),
                            start=(ci == 0), stop=(ci == 1),
                        )
                    nc.vector.tensor_scalar_add(
                        q_sb[:, co, ncols], pq[:], bqs[al][:, co:co + 1])

            # ---- v conv for one m-chunk (no bias; ACT copy to SBUF) ----
            def v_conv_chunk(a, al, vt, m):
                pv = pp_conv.tile([128, C], f32, tag="convp", name=f"pv{a}_{m}")
                for ci in range(2):
                    nc.tensor.matmul(
                        pv[:],
                        xsl(al, ci, slice(m * 128, (m + 1) * 128)),
                        wvs[al][:, ci, :],
                        start=(ci == 0), stop=(ci == 1),
                    )
                nc.scalar.activation(
                    vt[:, m, :], pv[:], mybir.ActivationFunctionType.Copy)

            # phase A for attention a (a >= 1): emitted during phB of a-1.
            def phase_a(a):
                al, be, ga = ATTN[a]
                scopeA = nc.named_scope(f"phA_{a}"); scopeA.__enter__()
                if a == 1:  # k0 not yet computed (k2, k1 done at startup)
                    for mt in range(NMT):
                        k_conv_chunk(0, mt)
                ksum = ksum_p.tile([128, 2, N], DT, tag="ksum", name=f"ksum{a}")
                ksum_add(ksum, be, ga, 0, NMT)
                q_sb = q_p.tile([128, 2, NH], DT, tag="q", name=f"q{a}")
                for nt in range(NNT):
                    q_conv_tile(a, al, q_sb, nt)
                vt = vt_p.tile([128, MC, C], DT, tag="vt", name=f"vt{a}")
                for m in range(MC):
                    v_conv_chunk(a, al, vt, m)
                scopeA.__exit__(None, None, None)
                return ksum, q_sb, vt

            # ---- output conv (interleaved: called per-nt once the last
            # attention's epilogue for that column range has flushed) ----
            def out_conv(nt):
                ncols = slice(nt * NT, (nt + 1) * NT)
                for co in range(2):
                    po = pp_conv.tile([128, NT], f32, tag="convp", name=f"po{nt}_{co}")
                    for ci in range(2):
                        nc.tensor.matmul(
                            po[:],
                            wo_sb[:, ci, co * 128:(co + 1) * 128],
                            fused[:, ci, ncols],
                            start=(ci == 0), stop=(ci == 1),
                        )
                    osb = out_p.tile([128, NT], f16, tag="osb", name=f"osb{nt}_{co}")
                    nc.vector.tensor_scalar_add(osb[:], po[:], bo_sb[:, co:co + 1])
                    nc.sync.dma_start(out=out_d[:, co, ncols], in_=osb[:])

            # ---- startup phase A for attention 0, pipelined with DMA ----
            al0, be0, ga0 = ATTN[0]
            scopeA0 = nc.named_scope("phA_0"); scopeA0.__enter__()
            ksum0 = ksum_p.tile([128, 2, N], DT, tag="ksum", name="ksum0")
            for mt in range(NMT):
                k_conv_chunk(be0, mt)   # k2
                k_conv_chunk(ga0, mt)   # k1
                ksum_add(ksum0, be0, ga0, mt, mt + 1)
            q0_sb = q_p.tile([128, 2, NH], DT, tag="q", name="q0")
            q_conv_tile(0, al0, q0_sb, 0)
            vt0 = vt_p.tile([128, MC, C], DT, tag="vt", name="vt0")
            # first few v chunks before scores start; rest interleave below
            for m in range(4):
                v_conv_chunk(0, al0, vt0, m)
            q_conv_tile(0, al0, q0_sb, 1)
            q_conv_tile(0, al0, q0_sb, 2)
            scopeA0.__exit__(None, None, None)
            v0_next = 4

            proj = (ksum0, q0_sb, vt0)
            for a, (al, be, ga) in enumerate(ATTN):
                ksum, q_sb, vt = proj
                # ---- phase B: attention over n-tiles ----
                for nt in range(NNT):
                    if nt == NNT - 1 and a < 2:
                        proj = phase_a(a + 1)
                    scopeB = nc.named_scope(f"phB_{a}_{nt}"); scopeB.__enter__()
                    flush_deferred()
                    if a == 2 and nt >= 1:
                        out_conv(nt - 1)
                    ncols = slice(nt * NT, (nt + 1) * NT)
                    at0 = pp_at.tile([128, NT], f32, tag="at0", name=f"at0_{a}_{nt}")
                    at1 = pp_at.tile([128, NT], f32, tag="at1", name=f"at1_{a}_{nt}")
                    dn = pp_dn.tile([1, NT], f32, tag="dn", name=f"dn{a}_{nt}")
                    # scores run LEAD chunks ahead of the accumulation
                    # passes: the lead hides the previous tile's epilogue
                    # chain (at-bank release) and the exp latency
                    LEAD = 6
                    pts = {}
                    for m in range(MC + LEAD):
                        if m < MC:
                            # attention 0 tile 0: interleave remaining v0
                            # conv chunks (LEAD ahead of the accum passes)
                            if a == 0 and nt == 0 and v0_next < MC and m >= 2:
                                v_conv_chunk(0, al0, vt0, v0_next)
                                v0_next += 1
                            sc_ps = pp_sc.tile([128, NT], f32, tag="sc", name=f"sc{a}_{nt}_{m}")
                            for ci in range(2):
                                nc.tensor.matmul(
                                    sc_ps[:],
                                    ksum[:, ci, m * 128:(m + 1) * 128],
                                    q_sb[:, ci, ncols],
                                    start=(ci == 0), stop=(ci == 1),
                                )
                            pt = pt_p.tile([128, NT], DT, tag="pt", name=f"pt{a}_{nt}_{m}")
                            nc.scalar.activation(
                                pt[:], sc_ps[:], mybir.ActivationFunctionType.Exp)
                            pts[m] = pt
                        if m >= LEAD:
                            j = m - LEAD
                            nc.tensor.matmul(at0[:], vt[:, j, 0:128], pts[j][:],
                                             start=(j == 0), stop=(j == MC - 1))
                            nc.tensor.matmul(at1[:], vt[:, j, 128:256], pts[j][:],
                                             start=(j == 0), stop=(j == MC - 1))
                            nc.tensor.matmul(dn[:], ones_col[:, j % 4:j % 4 + 1], pts[j][:],
                                             start=(j == 0), stop=(j == MC - 1))

                    # epilogue is deferred until after the NEXT scores pass
                    # is emitted, so the PE chews on those scores while this
                    # DVE/ACT chain drains; the at banks release just before
                    # the next tile's accumulation passes need them
                    r_sb = small_p.tile([1, NT], f32, tag="r", name=f"r{a}_{nt}")
                    nc.vector.reciprocal_approx_fast(r_sb[:], dn[:])

                    def epilogue(a=a, al=al, nt=nt, ncols=ncols, at0=at0,
                                 at1=at1, r_sb=r_sb):
                        rdt = small_p.tile([1, NT], DT, tag="rdt", name=f"rdt{a}_{nt}")
                        nc.vector.tensor_copy(rdt[:], r_sb[:])
                        # rb = 0.5 * recip(dn) broadcast to 128 partitions
                        rb = pp_conv.tile([128, NT], f32, tag="convp", name=f"rb{a}_{nt}")
                        nc.tensor.matmul(rb[:], half_row[:], rdt[:])
                        rb_sb = small_p.tile([128, NT], DT, tag="rb_sb", name=f"rbs{a}_{nt}")
                        nc.vector.tensor_copy(rb_sb[:], rb[:])
                        # attn_h = 0.5*attn = at * rb + 0.5*bv
                        attn = attn_p.tile([128, 2, NT], DT, tag="attn", name=f"attn{a}_{nt}")
                        nc.vector.tensor_mul(attn[:, 0, :], at0[:], rb_sb[:])
                        nc.vector.tensor_mul(attn[:, 1, :], at1[:], rb_sb[:])
                        for co in range(2):
                            nc.vector.tensor_scalar_add(
                                attn[:, co, :], attn[:, co, :],
                                bvh_sb[:, al, co:co + 1])
                        # gate: fused += attn_h + tanh(0.5*pg + bg/2)*attn_h
                        for co in range(2):
                            pg = pp_conv.tile([128, NT], f32, tag="convp", name=f"pg{a}_{nt}_{co}")
                            for ci in range(2):
                                nc.tensor.matmul(
                                    pg[:],
                                    wg_sb[:, ci, co * 128:(co + 1) * 128],
                                    attn[:, ci, :],
                                    start=(ci == 0), stop=(ci == 1),
                                )
                            th = eplg_p.tile([128, NT], DT, tag="th", name=f"th{a}_{nt}_{co}")
                            nc.scalar.activation(
                                th[:], pg[:], mybir.ActivationFunctionType.Tanh,
                                bias=bgh_sb[:, co:co + 1], scale=0.5)
                            gt = eplg_p.tile([128, NT], DT, tag="gt", name=f"gt{a}_{nt}_{co}")
                            nc.vector.tensor_mul(gt[:], th[:], attn[:, co, :])
                            nc.vector.tensor_add(
                                fused[:, co, ncols], fused[:, co, ncols],
                                attn[:, co, :])
                            nc.vector.tensor_add(
                                fused[:, co, ncols], fused[:, co, ncols], gt[:])

                    scopeB.__exit__(None, None, None)
                    deferred.append(epilogue)

            flush_deferred()
            out_conv(2)

    nc.compile()
    return nc


def _pack_bias(b):
    """[256] -> [128, 2] f32 (c_lo, c_hi)."""
    return np.ascontiguousarray(b.reshape(2, 128).T).astype(np.float32)


def _pack_w(w, scale=1.0):
    """[c_out, c_in] -> lhsT layout [128, 2, 256] = (c_in_lo, c_in_hi, c_out)."""
    wt = (w.astype(np.float64) * scale).astype(np.float32).T  # [c_in, c_out]
    return np.ascontiguousarray(
        wt.reshape(2, 128, C).transpose(1, 0, 2)).astype(_np_dt())


def _pack_chw(arr):
    """[256, X] f32 -> [128, 2, X] in DT order (c_lo, c_hi, X)."""
    return np.ascontiguousarray(
        arr.reshape(2, 128, -1).transpose(1, 0, 2)).astype(_np_dt())


def kernel(**inputs):
    global LAST_EXEC_NS, LAST_RESULTS
    inp = {k: np.asarray(v) for k, v in inputs.items()}
    s = inp["s"].astype(np.float32)

    if "nc" not in _CACHE:
        _CACHE["nc"] = build_bass()
    nc = _CACHE["nc"]

    # ---- host-side packing ----
    shared = {}
    for i in range(3):
        shared[f"wqT{i}"] = _pack_w(inp[f"wq{i}"], s[i] / SCALE)
        shared[f"wkT{i}"] = _pack_w(inp[f"wk{i}"], s[i])
        shared[f"wvT{i}"] = _pack_w(inp[f"wv{i}"])
        shared[f"bq{i}"] = _pack_bias(inp[f"bq{i}"].astype(np.float32) * (s[i] / SCALE))
        shared[f"bk{i}"] = _pack_bias(inp[f"bk{i}"].astype(np.float32) * s[i])
    shared["wgT"] = _pack_w(inp["wg"], 2.0)          # gate conv on halved attn
    shared["woT"] = _pack_w(inp["wo"])
    # 0.5*bv (added to the halved attn), per modality: [128, 3, 2]
    bvh = np.stack([_pack_bias(inp[f"bv{i}"].astype(np.float32) * 0.5)
                    for i in range(3)], axis=1)  # [128, 3, 2]
    shared["bvh"] = np.ascontiguousarray(bvh)
    shared["bgh"] = _pack_bias(inp["bg"].astype(np.float32) * 0.5)
    shared["bop"] = _pack_bias(inp["bo"])
    shared["ones_c"] = np.ones((128, 4), _np_dt())
    shared["half_r"] = np.full((1, 128), 0.5, _np_dt())

    in_maps = []
    for core in range(8):
        b, h = core // 2, core % 2
        m = dict(shared)
        for i in range(3):
            xp = _pack_chw(inp[f"x{i}"][b].reshape(C, N).astype(np.float32))
            if h == 1:  # rotate so this core's query half comes first
                xp = np.ascontiguousarray(
                    np.concatenate([xp[:, :, NH:], xp[:, :, :NH]], axis=2))
            m[f"x{i}"] = xp
        in_maps.append(m)

    trace = bool(os.environ.get("BASS_TRACE"))
    res = run_bass_kernel_spmd(nc, in_maps, core_ids=list(range(8)), trace=trace)
    LAST_EXEC_NS = res.exec_time_ns
    LAST_RESULTS = res

    out = np.empty((B, C, N), np.float32)
    for core in range(8):
        b, h = core // 2, core % 2
        o = np.asarray(res.results[core]["out"]).astype(np.float32)  # [128, 2, NH]
        out[b, :, h * NH:(h + 1) * NH] = o.transpose(1, 0, 2).reshape(C, NH)
    return out.reshape(B, C, H, W)


# revision 10
# speedup vs baseline: 1.2808x; 1.2494x over previous
"""Trainium2 Bass kernel for nn_CrossIQ (3-modality cross attention).

Reference computation (per batch b, with x0=rgb, x1=thermal, x2=depth):
    q_i = (wq_i @ x_i + bq_i) * s_i ; k_i likewise * s_i ; v_i = wv_i @ x_i + bv_i
    attend(q, ka, kb, v): softmax((q^T (ka+kb)) / 16, axis=m) -> a; out = v @ a^T
    w_rgb = attend(q0, k2, k1, v0); w_depth = attend(q2, k0, k1, v2)
    w_thermal = attend(q1, k0, k2, v1)
    fused = sum_i sigmoid(wg @ w_i + bg) * w_i ;  out = wo @ fused + bo

Sharding: 8 cores = (batch b in 0..3) x (query-half h in 0..1). Attention is
independent per batch and per query row n, so each core computes the full
k/v projections for its batch but only its half of the score rows.

v2 device dataflow per core (all matmul operands bf16, fp32 PSUM):
  - k_i[c,m] = wk_i @ x_i + bk_i*s_i computed ONCE per modality (72 PE passes
    instead of 144); ksum_a = k_be + k_ga via DVE pair-sum (2-byte 2x mode)
  - v bias is dropped on device: softmax weights sum to 1, so
    attend(q,ka,kb,v+bv) = attend(q,ka,kb,v) + bv. bv is re-added in the
    epilogue (0.5*bv onto the halved attn) and wg@bv is NOT needed because
    the epilogue adds bv before the gate conv.
  - scoresT[m,n] tiles = ksum^T q; pT = exp(scoresT) bf16 (max-free softmax)
  - attn_raw[c,n] += vT[m,:]^T @ pT ; denom[1,n] += ones^T @ pT
  - attn_h = attn_raw * (0.5*reciprocal(denom)) + 0.5*bv   (halved attention)
  - gate via tanh (same ACT table as Exp, so no 1.28us table reloads):
    sigma = 0.5*(1+tanh((wg@attn+bg)/2)); with wg2 = 2*wg on host,
    pg = wg2@attn_h = wg@attn, th = tanh(0.5*pg + bg/2)
    fused += attn_h + th*attn_h  ( = sigma * attn )
  - out = wo fused + bo, written fp16 (values <<1, fp16 exact to ~5e-4)
"""
import os
import sys
import types
import numpy as np
import ml_dtypes

# --- defensive shim: antenv.axon_hooks may be absent in this image; concourse
# imports it when trace=True under axon. Harmless no-op registration.
try:
    import antenv  # noqa: F401
    if "antenv.axon_hooks" not in sys.modules:
        _m = types.ModuleType("antenv.axon_hooks")
        _m._hook = None
        def _set(h):
            _m._hook = h
        def _get():
            return _m._hook
        _m.set_axon_ntff_profile_hook = _set
        _m.get_axon_ntff_profile_hook = _get
        sys.modules["antenv.axon_hooks"] = _m
        try:
            from trn_agent_boot.trn_boot import _ntff_profile_via_ctypes
            _h = _ntff_profile_via_ctypes("/opt/axon/libaxon_pjrt.so")
            if _h is not None:
                _m._hook = _h
        except Exception:
            pass
except Exception:
    pass

import concourse.bacc as bacc
import concourse.mybir as mybir
import concourse.tile as tile
from concourse.bass_utils import run_bass_kernel_spmd

B, C, H, W = 4, 256, 48, 48
N = H * W          # 2304 pixels (the m / key axis)
NH = N // 2        # 1152 query rows per core
NT = 384           # n-tile (<= 512 fp32 PSUM bank)
NNT = NH // NT     # 3 n-tiles
MC = N // 128      # 18 m-chunks
MT = 384           # m-tile for the k convs
NMT = N // MT      # 6
SCALE = 16.0       # sqrt(C)

DT_NAME = os.environ.get("KERNEL_DT", "bfloat16")

LAST_EXEC_NS = None
LAST_RESULTS = None

_CACHE = {}


def _dt():
    return getattr(mybir.dt, DT_NAME)


def _np_dt():
    return mybir.dt.np(_dt())


def build_bass():
    """Build the single-core program (identical on all 8 cores)."""
    DT = _dt()
    f32 = mybir.dt.float32
    f16 = mybir.dt.float16
    nc = bacc.Bacc("TRN2", target_bir_lowering=False, debug=False)

    # ---- DRAM I/O ----
    xs_d = [nc.dram_tensor(f"x{i}", [128, 2, N], DT, kind="ExternalInput").ap()
            for i in range(3)]
    wq_d = [nc.dram_tensor(f"wqT{i}", [128, 2, C], DT, kind="ExternalInput").ap()
            for i in range(3)]
    wk_d = [nc.dram_tensor(f"wkT{i}", [128, 2, C], DT, kind="ExternalInput").ap()
            for i in range(3)]
    wv_d = [nc.dram_tensor(f"wvT{i}", [128, 2, C], DT, kind="ExternalInput").ap()
            for i in range(3)]
    wg_d = nc.dram_tensor("wgT", [128, 2, C], DT, kind="ExternalInput").ap()
    wo_d = nc.dram_tensor("woT", [128, 2, C], DT, kind="ExternalInput").ap()
    bq_d = [nc.dram_tensor(f"bq{i}", [128, 2], f32, kind="ExternalInput").ap()
            for i in range(3)]
    bk_d = [nc.dram_tensor(f"bk{i}", [128, 2], f32, kind="ExternalInput").ap()
            for i in range(3)]
    bvh_d = nc.dram_tensor("bvh", [128, 3, 2], f32, kind="ExternalInput").ap()
    bgh_d = nc.dram_tensor("bgh", [128, 2], f32, kind="ExternalInput").ap()
    bo_d = nc.dram_tensor("bop", [128, 2], f32, kind="ExternalInput").ap()
    onc_d = nc.dram_tensor("two_sq", [128, 128], DT, kind="ExternalInput").ap()
    out_d = nc.dram_tensor("out", [128, 2, NH], f16, kind="ExternalOutput").ap()

    # attention spec: (alpha = q/v modality, beta/gamma = key modalities)
    ATTN = [(0, 2, 1), (2, 0, 1), (1, 0, 2)]

    with tile.TileContext(nc) as tc:
        with (
            tc.tile_pool(name="consts", bufs=1) as consts,
            tc.tile_pool(name="ksum_p", bufs=2) as ksum_p,
            tc.tile_pool(name="q_p", bufs=2) as q_p,
            tc.tile_pool(name="vt_p", bufs=2) as vt_p,
            tc.tile_pool(name="pt_p", bufs=8) as pt_p,
            tc.tile_pool(name="attn_p", bufs=1) as attn_p,
            tc.tile_pool(name="small_p", bufs=1) as small_p,
            tc.tile_pool(name="eplg_p", bufs=1) as eplg_p,
            tc.tile_pool(name="out_p", bufs=1) as out_p,
            tc.tile_pool(name="pp_conv", bufs=2, space="PSUM") as pp_conv,
            tc.tile_pool(name="pp_sc", bufs=3, space="PSUM") as pp_sc,
            tc.tile_pool(name="pp_at", bufs=1, space="PSUM") as pp_at,
            tc.tile_pool(name="pp_dn", bufs=1, space="PSUM") as pp_dn,
        ):
            # ---- load constants (small weights/biases first, then x in
            # chunks so the first convs can start early) ----
            def w_load(tag, src_ap, eng=None):
                eng = eng or nc.sync
                t = consts.tile([128, 2, C], DT, tag=tag, name=f"{tag}_sb")
                for st in range(2):
                    pr = slice(st * 64, (st + 1) * 64)
                    eng.dma_start(out=t[pr, :, :], in_=src_ap[pr, :, :])
                return t

            def b_load(tag, src_ap):
                t = consts.tile([128, 2], f32, tag=tag, name=f"{tag}_sb")
                nc.sync.dma_start(out=t[:], in_=src_ap[:])
                return t

            wks = [None, None, None]
            wqs = [None, None, None]
            wvs = [None, None, None]
            bqs = [None, None, None]
            bks = [None, None, None]
            x_tiles = {}
            for i in range(3):
                for ci in range(2):
                    x_tiles[(i, ci)] = consts.tile(
                        [128, N], DT, tag=f"x{i}_{ci}", name=f"x_sb{i}_{ci}")

            # k_i conv outputs, persistent (reused by 2 attentions each)
            k_sb = [consts.tile([128, 2, N], DT, tag=f"k{i}", name=f"k_sb{i}")
                    for i in range(3)]

            # priority: attention 0 needs ksum0 = k2+k1 (x2/x1) and q0 (x0
            # front cols). DMA x2/x1 in mt-granular chunks so the k convs
            # pipeline with the DMA; x0 follows.
            wks[2] = w_load("wk2", wk_d[2])
            wks[1] = w_load("wk1", wk_d[1])
            bks[2] = b_load("bk2", bk_d[2])
            bks[1] = b_load("bk1", bk_d[1])
            wqs[0] = w_load("wq0", wq_d[0])
            bqs[0] = b_load("bq0", bq_d[0])

            def x_load_chunk(i, ci, mt, eng):
                lo, hi = mt * MT, (mt + 1) * MT
                eng.dma_start(out=x_tiles[(i, ci)][:, lo:hi],
                              in_=xs_d[i][:, ci, lo:hi])

            for mt in range(NMT):
                x_load_chunk(2, 0, mt, nc.sync)
                x_load_chunk(2, 1, mt, nc.gpsimd)
                x_load_chunk(1, 0, mt, nc.sync)
                x_load_chunk(1, 1, mt, nc.gpsimd)
            wvs[0] = w_load("wv0", wv_d[0])
            for mt in range(NMT):
                x_load_chunk(0, 0, mt, nc.sync if mt % 2 else nc.gpsimd)
                x_load_chunk(0, 1, mt, nc.gpsimd if mt % 2 else nc.sync)
            wks[0] = w_load("wk0", wk_d[0], eng=nc.gpsimd)
            bks[0] = b_load("bk0", bk_d[0])
            for i in (1, 2):
                wqs[i] = w_load(f"wq{i}", wq_d[i], eng=nc.gpsimd)
                wvs[i] = w_load(f"wv{i}", wv_d[i], eng=nc.gpsimd)
                bqs[i] = b_load(f"bq{i}", bq_d[i])
            wg_sb = w_load("wg", wg_d, eng=nc.gpsimd)
            wo_sb = w_load("wo", wo_d, eng=nc.gpsimd)
            bvh_sb = consts.tile([128, 3, 2], f32, tag="bvh")
            nc.sync.dma_start(out=bvh_sb[:], in_=bvh_d[:])
            bgh_sb = consts.tile([128, 2], f32, tag="bgh")
            nc.sync.dma_start(out=bgh_sb[:], in_=bgh_d[:])
            bo_sb = consts.tile([128, 2], f32, tag="bo")
            nc.sync.dma_start(out=bo_sb[:], in_=bo_d[:])

            two_sq = consts.tile([128, 128], DT, tag="two_sq")
            nc.sync.dma_start(out=two_sq[:], in_=onc_d[:])

            def xsl(i, ci, cols):
                return x_tiles[(i, ci)][:, cols]

            deferred = []

            def flush_deferred():
                while deferred:
                    deferred.pop(0)()

            fused = consts.tile([128, 2, NH], DT, tag="fused")
            nc.vector.memset(fused[:], 0.0)

            # ---- modality k conv: k_i[:, co, mt-cols] (+bias via DVE) ----
            def k_conv_chunk(i, mt):
                mcols = slice(mt * MT, (mt + 1) * MT)
                for co in range(2):
                    pk = pp_conv.tile([128, MT], f32, tag="convp",
                                      name=f"pk{i}_{co}_{mt}")
                    for ci in range(2):
                        nc.tensor.matmul(
                            pk[:],
                            wks[i][:, ci, co * 128:(co + 1) * 128],
                            xsl(i, ci, mcols),
                            start=(ci == 0), stop=(ci == 1),
                        )
                    nc.vector.tensor_scalar_add(
                        k_sb[i][:, co, mcols], pk[:], bks[i][:, co:co + 1])

            # ---- pair sum ksum_a = k_be + k_ga (DVE, bf16 2x mode) ----
            def ksum_add(ksum, be, ga, mt_lo, mt_hi):
                mcols = slice(mt_lo * MT, mt_hi * MT)
                nc.vector.tensor_add(
                    ksum[:, :, mcols], k_sb[be][:, :, mcols],
                    k_sb[ga][:, :, mcols])

            # ---- q conv for one n-tile ----
            def q_conv_tile(a, al, q_sb, nt):
                ncols = slice(nt * NT, (nt + 1) * NT)
                for co in range(2):
                    pq = pp_conv.tile([128, NT], f32, tag="convp",
                                      name=f"pq{a}_{co}_{nt}")
                    for ci in range(2):
                        nc.tensor.matmul(
                            pq[:],
                            wqs[al][:, ci, co * 128:(co + 1) * 128],
                            xsl(al, ci, ncols),
                            start=(ci == 0), stop=(ci == 1),
                        )
                    nc.vector.tensor_scalar_add(
                        q_sb[:, co, ncols], pq[:], bqs[al][:, co:co + 1])

            # ---- v conv for one m-chunk (no bias; ACT copy to SBUF) ----
            def v_conv_chunk(a, al, vt, m):
                pv = pp_conv.tile([128, C], f32, tag="convp", name=f"pv{a}_{m}")
                for ci in range(2):
                    nc.tensor.matmul(
                        pv[:],
                        xsl(al, ci, slice(m * 128, (m + 1) * 128)),
                        wvs[al][:, ci, :],
                        start=(ci == 0), stop=(ci == 1),
                    )
                nc.vector.tensor_copy(vt[:, m, :], pv[:])

            # phase A for attention a (a >= 1): emitted during phB of a-1.
            def phase_a(a):
                al, be, ga = ATTN[a]
                scopeA = nc.named_scope(f"phA_{a}"); scopeA.__enter__()
                if a == 1:  # k0 not yet computed (k2, k1 done at startup)
                    for mt in range(NMT):
                        k_conv_chunk(0, mt)
                ksum = ksum_p.tile([128, 2, N], DT, tag="ksum", name=f"ksum{a}")
                ksum_add(ksum, be, ga, 0, NMT)
                q_sb = q_p.tile([128, 2, NH], DT, tag="q", name=f"q{a}")
                for nt in range(NNT):
                    q_conv_tile(a, al, q_sb, nt)
                vt = vt_p.tile([128, MC, C], DT, tag="vt", name=f"vt{a}")
                for m in range(MC):
                    v_conv_chunk(a, al, vt, m)
                scopeA.__exit__(None, None, None)
                return ksum, q_sb, vt

            # ---- output conv (interleaved: called per-nt once the last
            # attention's epilogue for that column range has flushed) ----
            def out_conv(nt):
                ncols = slice(nt * NT, (nt + 1) * NT)
                for co in range(2):
                    po = pp_conv.tile([128, NT], f32, tag="convp", name=f"po{nt}_{co}")
                    for ci in range(2):
                        nc.tensor.matmul(
                            po[:],
                            wo_sb[:, ci, co * 128:(co + 1) * 128],
                            fused[:, ci, ncols],
                            start=(ci == 0), stop=(ci == 1),
                        )
                    osb = out_p.tile([128, NT], f16, tag="osb", name=f"osb{nt}_{co}")
                    nc.vector.tensor_scalar_add(osb[:], po[:], bo_sb[:, co:co + 1])
                    nc.sync.dma_start(out=out_d[:, co, ncols], in_=osb[:])

            # ---- startup phase A for attention 0, pipelined with DMA ----
            al0, be0, ga0 = ATTN[0]
            scopeA0 = nc.named_scope("phA_0"); scopeA0.__enter__()
            ksum0 = ksum_p.tile([128, 2, N], DT, tag="ksum", name="ksum0")
            for mt in range(NMT):
                k_conv_chunk(be0, mt)   # k2
                k_conv_chunk(ga0, mt)   # k1
                ksum_add(ksum0, be0, ga0, mt, mt + 1)
            q0_sb = q_p.tile([128, 2, NH], DT, tag="q", name="q0")
            q_conv_tile(0, al0, q0_sb, 0)
            vt0 = vt_p.tile([128, MC, C], DT, tag="vt", name="vt0")
            # first few v chunks before scores start; rest interleave below
            for m in range(4):
                v_conv_chunk(0, al0, vt0, m)
            q_conv_tile(0, al0, q0_sb, 1)
            q_conv_tile(0, al0, q0_sb, 2)
            scopeA0.__exit__(None, None, None)
            v0_next = 4

            proj = (ksum0, q0_sb, vt0)
            for a, (al, be, ga) in enumerate(ATTN):
                ksum, q_sb, vt = proj
                # ---- phase B: attention over n-tiles ----
                for nt in range(NNT):
                    if nt == NNT - 1 and a < 2:
                        proj = phase_a(a + 1)
                    scopeB = nc.named_scope(f"phB_{a}_{nt}"); scopeB.__enter__()
                    flush_deferred()
                    if a == 2 and nt >= 1:
                        out_conv(nt - 1)
                    ncols = slice(nt * NT, (nt + 1) * NT)
                    at0 = pp_at.tile([128, NT], f32, tag="at0", name=f"at0_{a}_{nt}")
                    at1 = pp_at.tile([128, NT], f32, tag="at1", name=f"at1_{a}_{nt}")
                    # dn accumulates 2*sum_m pt on ALL 128 partitions (lhsT
                    # is a [128,128] all-2.0 matrix: same matmul cost, but
                    # the denominator lands pre-broadcast, so the epilogue
                    # needs no ones-row broadcast matmul / copies, and the
                    # 2.0 folds the 0.5 attention halving into the recip)
                    dn = pp_dn.tile([128, NT], f32, tag="dn", name=f"dn{a}_{nt}")
                    # scores run LEAD chunks ahead of the accumulation
                    # passes: the lead hides the previous tile's epilogue
                    # chain (at-bank release) and the exp latency
                    LEAD = 6
                    pts = {}
                    for m in range(MC + LEAD):
                        if m < MC:
                            # attention 0 tile 0: interleave remaining v0
                            # conv chunks (LEAD ahead of the accum passes)
                            if a == 0 and nt == 0 and v0_next < MC and m >= 2:
                                v_conv_chunk(0, al0, vt0, v0_next)
                                v0_next += 1
                            sc_ps = pp_sc.tile([128, NT], f32, tag="sc", name=f"sc{a}_{nt}_{m}")
                            for ci in range(2):
                                nc.tensor.matmul(
                                    sc_ps[:],
                                    ksum[:, ci, m * 128:(m + 1) * 128],
                                    q_sb[:, ci, ncols],
                                    start=(ci == 0), stop=(ci == 1),
                                )
                            pt = pt_p.tile([128, NT], DT, tag="pt", name=f"pt{a}_{nt}_{m}")
                            nc.scalar.activation(
                                pt[:], sc_ps[:], mybir.ActivationFunctionType.Exp)
                            pts[m] = pt
                        if m >= LEAD:
                            j = m - LEAD
                            nc.tensor.matmul(at0[:], vt[:, j, 0:128], pts[j][:],
                                             start=(j == 0), stop=(j == MC - 1))
                            nc.tensor.matmul(at1[:], vt[:, j, 128:256], pts[j][:],
                                             start=(j == 0), stop=(j == MC - 1))
                            nc.tensor.matmul(dn[:], two_sq[:], pts[j][:],
                                             start=(j == 0), stop=(j == MC - 1))

                    # epilogue is deferred until after the NEXT scores pass
                    # is emitted, so the PE chews on those scores while this
                    # DVE/ACT chain drains; the at banks release just before
                    # the next tile's accumulation passes need them
                    # r = 0.5/sum_m pt (the 2.0 lhsT folds the halving),
                    # already broadcast on all 128 partitions
                    r_sb = small_p.tile([128, NT], f32, tag="r", name=f"r{a}_{nt}")
                    nc.vector.reciprocal_approx_fast(r_sb[:], dn[:])

                    def epilogue(a=a, al=al, nt=nt, ncols=ncols, at0=at0,
                                 at1=at1, r_sb=r_sb):
                        # attn_h = 0.5*attn = at * r + 0.5*bv
                        attn = attn_p.tile([128, 2, NT], DT, tag="attn", name=f"attn{a}_{nt}")
                        nc.vector.tensor_mul(attn[:, 0, :], at0[:], r_sb[:])
                        nc.vector.tensor_mul(attn[:, 1, :], at1[:], r_sb[:])
                        for co in range(2):
                            nc.vector.tensor_scalar_add(
                                attn[:, co, :], attn[:, co, :],
                                bvh_sb[:, al, co:co + 1])
                        # gate: fused += attn_h + tanh(0.5*pg + bg/2)*attn_h
                        for co in range(2):
                            pg = pp_conv.tile([128, NT], f32, tag="convp", name=f"pg{a}_{nt}_{co}")
                            for ci in range(2):
                                nc.tensor.matmul(
                                    pg[:],
                                    wg_sb[:, ci, co * 128:(co + 1) * 128],
                                    attn[:, ci, :],
                                    start=(ci == 0), stop=(ci == 1),
                                )
                            th = eplg_p.tile([128, NT], DT, tag="th", name=f"th{a}_{nt}_{co}")
                            nc.scalar.activation(
                                th[:], pg[:], mybir.ActivationFunctionType.Tanh,
                                bias=bgh_sb[:, co:co + 1], scale=0.5)
                            gt = eplg_p.tile([128, NT], DT, tag="gt", name=f"gt{a}_{nt}_{co}")
                            nc.vector.tensor_mul(gt[:], th[:], attn[:, co, :])
                            nc.vector.tensor_add(
                                fused[:, co, ncols], fused[:, co, ncols],
                                attn[:, co, :])
                            nc.vector.tensor_add(
                                fused[:, co, ncols], fused[:, co, ncols], gt[:])

                    scopeB.__exit__(None, None, None)
                    deferred.append(epilogue)

            flush_deferred()
            out_conv(2)

    nc.compile()
    return nc


def _pack_bias(b):
    """[256] -> [128, 2] f32 (c_lo, c_hi)."""
    return np.ascontiguousarray(b.reshape(2, 128).T).astype(np.float32)


def _pack_w(w, scale=1.0):
    """[c_out, c_in] -> lhsT layout [128, 2, 256] = (c_in_lo, c_in_hi, c_out)."""
    wt = (w.astype(np.float64) * scale).astype(np.float32).T  # [c_in, c_out]
    return np.ascontiguousarray(
        wt.reshape(2, 128, C).transpose(1, 0, 2)).astype(_np_dt())


def _pack_chw(arr):
    """[256, X] f32 -> [128, 2, X] in DT order (c_lo, c_hi, X)."""
    return np.ascontiguousarray(
        arr.reshape(2, 128, -1).transpose(1, 0, 2)).astype(_np_dt())


def kernel(**inputs):
    global LAST_EXEC_NS, LAST_RESULTS
    inp = {k: np.asarray(v) for k, v in inputs.items()}
    s = inp["s"].astype(np.float32)

    if "nc" not in _CACHE:
        _CACHE["nc"] = build_bass()
    nc = _CACHE["nc"]

    # ---- host-side packing ----
    shared = {}
    for i in range(3):
        shared[f"wqT{i}"] = _pack_w(inp[f"wq{i}"], s[i] / SCALE)
        shared[f"wkT{i}"] = _pack_w(inp[f"wk{i}"], s[i])
        shared[f"wvT{i}"] = _pack_w(inp[f"wv{i}"])
        shared[f"bq{i}"] = _pack_bias(inp[f"bq{i}"].astype(np.float32) * (s[i] / SCALE))
        shared[f"bk{i}"] = _pack_bias(inp[f"bk{i}"].astype(np.float32) * s[i])
    shared["wgT"] = _pack_w(inp["wg"], 2.0)          # gate conv on halved attn
    shared["woT"] = _pack_w(inp["wo"])
    # 0.5*bv (added to the halved attn), per modality: [128, 3, 2]
    bvh = np.stack([_pack_bias(inp[f"bv{i}"].astype(np.float32) * 0.5)
                    for i in range(3)], axis=1)  # [128, 3, 2]
    shared["bvh"] = np.ascontiguousarray(bvh)
    shared["bgh"] = _pack_bias(inp["bg"].astype(np.float32) * 0.5)
    shared["bop"] = _pack_bias(inp["bo"])
    shared["two_sq"] = np.full((128, 128), 2.0, _np_dt())

    in_maps = []
    for core in range(8):
        b, h = core // 2, core % 2
        m = dict(shared)
        for i in range(3):
            xp = _pack_chw(inp[f"x{i}"][b].reshape(C, N).astype(np.float32))
            if h == 1:  # rotate so this core's query half comes first
                xp = np.ascontiguousarray(
                    np.concatenate([xp[:, :, NH:], xp[:, :, :NH]], axis=2))
            m[f"x{i}"] = xp
        in_maps.append(m)

    trace = bool(os.environ.get("BASS_TRACE"))
    res = run_bass_kernel_spmd(nc, in_maps, core_ids=list(range(8)), trace=trace)
    LAST_EXEC_NS = res.exec_time_ns
    LAST_RESULTS = res

    out = np.empty((B, C, N), np.float32)
    for core in range(8):
        b, h = core // 2, core % 2
        o = np.asarray(res.results[core]["out"]).astype(np.float32)  # [128, 2, NH]
        out[b, :, h * NH:(h + 1) * NH] = o.transpose(1, 0, 2).reshape(C, NH)
    return out.reshape(B, C, H, W)
